# revision 34
# baseline (speedup 1.0000x reference)
"""Multi-head attention TRN2 kernel (Bass/Tile), 8-core tensor-parallel.

Sharding: core c -> batch b=c//4, head group g=c%4 (4 heads = 256 features).
Host pre-transposes x and weight slices to bf16 and RELAYS them so each
SBUF partition's slice is one contiguous DRAM run (the DMA engines here
are descriptor-rate-bound, not bandwidth-bound): x as [128p, ts, e, 512]
(8KB/partition/slice), weights as [128p, e, 256] (4KB/partition).

Device computes qT/kT (features x tokens) and v in token-major
[v(64)|ones(64)] blocks per head, so each AV matmul emits the softmax
denominator replicated across psum partitions 64:128, row-aligned with the
context rows.  Causal softmax uses unnormalized exp (scalar engine) with
per-chunk column trimming at the diagonal; 1/denominator is exp(-ln(x)) on
the scalar engine, then one vector multiply.  The output projection
partial goes back fp32; the host sums 4 partials per batch and adds bias.

All matmul operands are bf16 (fp32 psum accumulation).  Pipeline: per
512-token slice: projections -> attention (heads paired per feature tile,
AV of group gi-1 issued between QK and exp of gi) -> output projection of
slice ts-1 emitted after attention(ts) so its matmuls fill the
activation-bound attention tail; stores batched [128,1024] and rotated
across the sync/scalar/gpsimd DMA queues so they drain during compute.
"""

import numpy as np

B, S, D = 2, 2048, 1024
H, HD = 16, 64
NCORES = 8
HPC = 4              # heads per core
FPC = HPC * HD       # 256 features per core
NF = FPC // 128      # 2 feature tiles of 128
KC = D // 128        # 8 contraction chunks
NTS = S // 512       # 4 token slices == q-tiles
NTT = S // 128       # 16 token tiles of 128
SCALE = 1.0 / 8.0    # 1/sqrt(HD)

_CACHE = {}


def _legalize_waits(nc, mybir, max_waits=1):
    """Walrus codegen allows only 1 sync-wait slot on most TPB instructions.
    Hoist extra waits into same-engine NoOps inserted just before."""
    n_fixed = 0
    for _, bb_wrap in nc.bb_map.items():
        bb = bb_wrap.bb
        out = []
        changed = False
        for inst in list(bb.instructions):
            si = inst.sync_info
            if si is not None and si.on_wait and len(si.on_wait) > max_waits:
                for w in list(si.on_wait[:-max_waits]):
                    nop = mybir.InstNoOp(
                        name=f"I-lw-{nc.next_id()}", engine=inst.engine,
                        ins=[], outs=[],
                        sync_info=mybir.SyncInfo(on_wait=[w], on_update=[]),
                    )
                    nop.text_hint = "dep"
                    out.append(nop)
                si.on_wait = list(si.on_wait[-max_waits:])
                n_fixed += 1
                changed = True
            out.append(inst)
        if changed:
            bb.instructions = out
    return n_fixed


def _build():
    import concourse.bass as bass
    import concourse.mybir as mybir
    from concourse.tile import TileContext
    from concourse.masks import make_upper_triangular

    F32 = mybir.dt.float32
    BF16 = mybir.dt.bfloat16
    EXP = mybir.ActivationFunctionType.Exp
    LN = mybir.ActivationFunctionType.Ln
    MUL = mybir.AluOpType.mult

    nc = bass.Bass()
    # host-relaid: [128p, ts, e, 512] flattened
    xT = nc.dram_tensor("xT", [128, NTS * KC * 512], BF16,
                        kind="ExternalInput")
    # host-relaid: [128p, e, 256] flattened
    wqT = nc.dram_tensor("wqT", [128, KC * FPC], BF16, kind="ExternalInput")
    wkT = nc.dram_tensor("wkT", [128, KC * FPC], BF16, kind="ExternalInput")
    wvT = nc.dram_tensor("wvT", [128, KC * FPC], BF16, kind="ExternalInput")
    woT = nc.dram_tensor("woT", [FPC, D], BF16, kind="ExternalInput")
    outp = nc.dram_tensor("outp", [S, D], BF16, kind="ExternalOutput")

    with TileContext(nc) as tc:
        with (
            tc.tile_pool(name="res", bufs=1) as res,
            tc.tile_pool(name="xp", bufs=2) as xp,
            tc.tile_pool(name="wk", bufs=4) as wkp,
            tc.tile_pool(name="osb", bufs=6) as osb,
            tc.tile_pool(name="pjps", bufs=1, space="PSUM") as pjps,
            tc.tile_pool(name="qkps", bufs=2, space="PSUM") as qkps,
            tc.tile_pool(name="smps", bufs=1, space="PSUM") as smps,
            tc.tile_pool(name="avps", bufs=1, space="PSUM") as avps,
        ):
            # ---- resident tensors -------------------------------------
            qT = [res.tile([128, S], BF16, name=f"qT{f}", tag=f"qT{f}")
                  for f in range(NF)]
            kT = [res.tile([128, S], BF16, name=f"kT{f}", tag=f"kT{f}")
                  for f in range(NF)]
            ctxT = [res.tile([128, S], BF16, name=f"ctxT{f}", tag=f"ctxT{f}")
                    for f in range(NF)]
            # v chunk layout: per head hh a [v(64) | ones(64)] block, so the
            # AV matmul emits the softmax denominator replicated over
            # partitions 64:128 of its psum bank.
            v_sb = [res.tile([128, 512], BF16, name=f"v{ck}", tag=f"v{ck}")
                    for ck in range(NTT)]
            woT_sb = [res.tile([128, D], BF16, name=f"wo{ic}", tag=f"wo{ic}")
                      for ic in range(NF)]

            # PE warm-up: a dense burst of dummy matmuls during the initial
            # DMA wait keeps the PE p-state at full rate until the first
            # projection group lands.  memset on the (empty) vector queue so
            # the burst starts before the DMA programming below.
            warm_sb = res.tile([128, 512], BF16, name="warm")
            nc.vector.memset(warm_sb, 0.0)
            wps = pjps.tile([128, 512], F32, name="pqk", tag="sm")
            for wi in range(32):
                c0 = 256 * (wi % 2)
                nc.tensor.matmul(wps[:, c0:c0 + 256], warm_sb[:, 0:128],
                                 warm_sb[:, 0:256], start=True, stop=True)

            # Batched loads: one DMA per tensor slice; the host layout makes
            # each partition's slice a single contiguous DRAM run, so the
            # descriptor count per load is 128 (vs 1k+ for strided views).
            _dmae = [nc.sync, nc.scalar, nc.gpsimd]
            # store/x-load queues: never scalar (it owns the exp stream)
            _dmst = [nc.sync, nc.gpsimd]

            def load_x(ts):
                t = xp.tile([128, KC * 512], BF16, name="x", tag="x")
                _dmst[ts % 2].dma_start(
                    out=t, in_=xT[:, ts * KC * 512:(ts + 1) * KC * 512])
                return [t[:, e * 512:(e + 1) * 512] for e in range(KC)]

            w_t = {}

            def load_w(nm, dram, eng):
                t = wkp.tile([128, KC * FPC], BF16, name=f"w{nm}",
                             tag=f"w{nm}", bufs=1)
                eng.dma_start(out=t, in_=dram[:, :])
                for e in range(KC):
                    w_t[nm, e] = t[:, e * FPC:(e + 1) * FPC]

            xch0 = load_x(0)
            load_w("q", wqT, nc.scalar)
            load_w("k", wkT, nc.gpsimd)
            load_w("v", wvT, nc.sync)

            # constants
            ones_f = res.tile([128, 64], F32)
            nc.gpsimd.memset(ones_f, 1.0)
            mask_f = res.tile([128, 128], F32)
            make_upper_triangular(nc, mask_f, val=1.0, diag=True)
            mask_r = res.tile([128, 128], BF16)
            nc.vector.tensor_copy(mask_r, mask_f)
            for ck in range(NTT):
                v4 = v_sb[ck].rearrange("p (h c) -> p h c", c=128)
                for hh in range(HPC):
                    nc.vector.tensor_copy(v4[:, hh, 64:128], ones_f)

            for ic in range(NF):
                nc.sync.dma_start(
                    out=woT_sb[ic],
                    in_=woT[ic * 128:(ic + 1) * 128, :])

            # Output-projection groups double as PE bubble fillers: their
            # matmuls depend only on already-normalized ctx slices, so they
            # are emitted into the attention's per-f pipeline-fill/drain
            # bubbles.  Stores are batched per token tile ([128,1024] = one
            # contiguous 2KB run per partition) and rotated across the
            # sync/scalar/gpsimd DMA queues so they drain during compute.
            def op_groups(tts, pools, eng=None):
                for gidx, tt in enumerate(tts):
                    so = osb.tile([128, D], BF16, name="so", tag="so")
                    for os_ in range(2):
                        pool, tg = pools[(gidx * 2 + os_) % len(pools)]
                        po = pool.tile([128, 512], F32, name="po", tag=tg)
                        for ic in range(NF):
                            nc.tensor.matmul(
                                po,
                                ctxT[ic][:, tt * 128:(tt + 1) * 128],
                                woT_sb[ic][:, os_ * 512:(os_ + 1) * 512],
                                start=(ic == 0), stop=(ic == NF - 1))
                        nc.vector.tensor_copy(
                            so[:, os_ * 512:(os_ + 1) * 512], po)
                    (eng or nc.sync).dma_start(
                        out=outp[tt * 128:(tt + 1) * 128, :], in_=so)

            MIDP = [(smps, "po"), (avps, "av")]
            ENDP = [(smps, "po"), (qkps, "qk")]

            # ---- pipelined slices -------------------------------------
            for ts in range(NTS):
                xch = xch0 if ts == 0 else load_x(ts)

                # projections for this slice; psum alternates between the
                # pjps and smps banks so the next group's matmuls overlap
                # the previous group's psum->sbuf drain.
                _pj = [(pjps, "sm"), (smps, "po")]
                pji = 0
                for nm, dst in (("q", qT), ("k", kT)):
                    for f in range(NF):
                        pool, tg = _pj[pji % 2]
                        pji += 1
                        ps = pool.tile([128, 512], F32, name="pqk", tag=tg)
                        for e in range(KC):
                            nc.tensor.matmul(
                                ps, w_t[nm, e][:, f * 128:(f + 1) * 128],
                                xch[e], start=(e == 0), stop=(e == KC - 1))
                        nc.vector.tensor_copy(
                            dst[f][:, ts * 512:(ts + 1) * 512], ps)
                for tt in range(4):
                    ck = ts * 4 + tt
                    pool, tg = _pj[pji % 2]
                    pji += 1
                    ps = pool.tile([128, FPC], F32, name="pv", tag=tg)
                    for e in range(KC):
                        nc.tensor.matmul(
                            ps, xch[e][:, tt * 128:(tt + 1) * 128],
                            w_t["v", e], start=(e == 0), stop=(e == KC - 1))
                    v4 = v_sb[ck].rearrange("p (h c) -> p h c", c=128)
                    ps4 = ps.rearrange("p (g c) -> p g c", c=64)
                    nc.vector.tensor_copy(v4[:, :, 0:64], ps4)

                # attention for q-tile j == ts.  The two heads of each
                # feature tile f run as a row-tiled pair: head hl=0 on PE
                # rows 0:64 (tile_position (0,0)), hl=1 on rows 64:128
                # ((64,0)) — concurrent on the array, halving QK^T time.
                # AV matmuls of group gi-1 are issued between the QK pair
                # of gi and its exp so the PE stays busy during the
                # activation.
                j = ts
                sq0 = 512 * j
                ndiag = 4 * j  # first diagonal chunk index

                def qk_trim(ci):
                    """Leading fully-masked columns to skip."""
                    return max(128 * ci - sq0, 0)

                for f in range(NF):
                    av = avps.tile([128, 1024], F32, name="av", tag="av")
                    ag_prev = None

                    def av_mms(ag_pair, gi):
                        for slot in range(2):
                            ci = 2 * gi + slot
                            tr = qk_trim(ci)
                            for hl in range(2):
                                hh = 2 * f + hl
                                nc.tensor.matmul(
                                    av[:, 512 * hl + tr:512 * hl + 512],
                                    v_sb[ci][:, 128 * hh:128 * hh + 128],
                                    ag_pair[hl][:, slot * 512 + tr:
                                                 slot * 512 + 512],
                                    start=(ci == 0),
                                    stop=(ci == 4 * j + 3))

                    for gi in range(2 * j + 2):
                        qk_pair = [qkps.tile([128, 1024], F32, name="qk",
                                             tag="qk") for _ in range(2)]
                        for slot in range(2):
                            ci = 2 * gi + slot
                            tr = qk_trim(ci)
                            for hl in range(2):
                                r0 = 64 * hl
                                nc.tensor.matmul(
                                    qk_pair[hl][:, slot * 512 + tr:
                                                slot * 512 + 512],
                                    kT[f][r0:r0 + 64,
                                          ci * 128:(ci + 1) * 128],
                                    qT[f][r0:r0 + 64,
                                          sq0 + tr:sq0 + 512],
                                    start=True, stop=True)
                        if ag_prev is not None:
                            av_mms(ag_prev, gi - 1)
                        ag_pair = [osb.tile([128, 1024], BF16, name="ag",
                                            tag="ag", bufs=6)
                                   for _ in range(2)]
                        for hl in range(2):
                            qk, ag = qk_pair[hl], ag_pair[hl]
                            if gi < 2 * j:  # fully-valid chunks
                                nc.scalar.activation(ag, qk, EXP,
                                                     scale=SCALE)
                            else:
                                # diagonal: one exp from the first valid
                                # column onward (cols the av matmuls never
                                # read may hold garbage), then triangular
                                # masks per 128-chunk.
                                tr0 = qk_trim(2 * gi)
                                nc.scalar.activation(
                                    ag[:, tr0:1024], qk[:, tr0:1024],
                                    EXP, scale=SCALE)
                                for slot in range(2):
                                    ci = 2 * gi + slot
                                    dlt = 128 * ci - sq0
                                    w0 = slot * 512
                                    nc.vector.tensor_tensor(
                                        ag[:, w0 + dlt:w0 + dlt + 128],
                                        ag[:, w0 + dlt:w0 + dlt + 128],
                                        mask_r, MUL)
                        ag_prev = ag_pair
                    av_mms(ag_prev, 2 * j + 1)

                    # normalize: ctxT = av[0:64] * (1/av[64:128]); the ones
                    # half of each v block put the denominator in av rows
                    # 64:128 (replicated), row-aligned with ctx.  1/x is
                    # exp(-ln(x)) on the scalar engine: ln and exp share one
                    # activation table.  Both heads batched per call.
                    lns = osb.tile([64, 1024], F32, name="lns", tag="lns")
                    nc.scalar.activation(lns, av[64:128, :], LN)
                    rcp = osb.tile([64, 1024], F32, name="rcp", tag="rcp")
                    nc.scalar.activation(rcp, lns, EXP, scale=-1.0)
                    for hl in range(2):
                        r0 = 64 * hl
                        nc.vector.tensor_tensor(
                            ctxT[f][r0:r0 + 64, sq0:sq0 + 512],
                            av[0:64, 512 * hl:512 * hl + 512],
                            rcp[:, 512 * hl:512 * hl + 512], MUL)

                    # mid-slice filler: cover the f=0 drain / f=1 fill
                    # bubble with ready out-projection groups.
                    if f == 0 and ts >= 1:
                        op_groups([4 * (ts - 1), 4 * (ts - 1) + 1], MIDP)

                # end-of-slice filler: cover the f=1 drain bubble.
                if ts >= 2:
                    op_groups([4 * (ts - 1) + 2, 4 * (ts - 1) + 3], ENDP)

            # Tail: op0's second half is dependency-free PE work that runs
            # while the final normalize chain (scalar ln/exp + vector
            # muls) drains; then op3, whose ic=1 matmuls are emitted after
            # all ic=0 so the PE chews on those first if ctxT[1] is late.
            op_groups([2, 3], ENDP)
            op3_pools = [(smps, "po"), (pjps, "sm"), (qkps, "qk"),
                         (qkps, "qk")]
            for wave in range(2):
                tts = [12 + 2 * wave, 13 + 2 * wave]
                pos = {}
                for wi, tt in enumerate(tts):
                    for os_ in range(2):
                        pool, tg = op3_pools[(2 * wi + os_) % 4]
                        po = pool.tile([128, 512], F32, name="po", tag=tg)
                        pos[tt, os_] = po
                        nc.tensor.matmul(
                            po, ctxT[0][:, tt * 128:(tt + 1) * 128],
                            woT_sb[0][:, os_ * 512:(os_ + 1) * 512],
                            start=True, stop=False)
                for tt in tts:
                    so = osb.tile([128, D], BF16, name="so", tag="so")
                    for os_ in range(2):
                        po = pos[tt, os_]
                        nc.tensor.matmul(
                            po, ctxT[1][:, tt * 128:(tt + 1) * 128],
                            woT_sb[1][:, os_ * 512:(os_ + 1) * 512],
                            start=False, stop=True)
                        nc.vector.tensor_copy(
                            so[:, os_ * 512:(os_ + 1) * 512], po)
                    nc.sync.dma_start(
                        out=outp[tt * 128:(tt + 1) * 128, :], in_=so)

    _legalize_waits(nc, mybir)
    return nc


def _prep_inputs(in_data, Wq, Wk, Wv, Wo):
    import ml_dtypes

    bf16 = ml_dtypes.bfloat16
    in_maps = []
    for c in range(NCORES):
        b, g = c // 4, c % 4
        sl = slice(g * FPC, (g + 1) * FPC)
        # x: [D, S] -> [128p, ts, e, 512] so each (p, ts) is one
        # contiguous 8KB DRAM run.
        xt = np.ascontiguousarray(in_data[b].T).astype(bf16)
        xr = np.ascontiguousarray(
            xt.reshape(KC, 128, NTS, 512).transpose(1, 2, 0, 3)
        ).reshape(128, NTS * KC * 512)

        def wrelay(W):
            # [D, FPC] -> [128p, e, FPC] contiguous per partition
            wt = np.ascontiguousarray(W[sl, :].T).astype(bf16)
            return np.ascontiguousarray(
                wt.reshape(KC, 128, FPC).transpose(1, 0, 2)
            ).reshape(128, KC * FPC)

        in_maps.append({
            "xT": xr,
            "wqT": wrelay(Wq),
            "wkT": wrelay(Wk),
            "wvT": wrelay(Wv),
            "woT": np.ascontiguousarray(Wo[:, sl].T).astype(bf16),
        })
    return in_maps


def run(inputs, trace=False):
    from concourse.bass_utils import run_bass_kernel_spmd

    in_data = np.asarray(inputs["in_data"], dtype=np.float32)
    Wq = np.asarray(inputs["Wq"], dtype=np.float32)
    Wk = np.asarray(inputs["Wk"], dtype=np.float32)
    Wv = np.asarray(inputs["Wv"], dtype=np.float32)
    Wo = np.asarray(inputs["Wo"], dtype=np.float32)
    bo = np.asarray(inputs["bo"], dtype=np.float32)

    if "nc" not in _CACHE:
        _CACHE["nc"] = _build()
    nc = _CACHE["nc"]

    in_maps = _prep_inputs(in_data, Wq, Wk, Wv, Wo)
    kw = {}
    if trace:
        kw = dict(trace=True, trace_cores=list(range(NCORES)))
    try:
        res = run_bass_kernel_spmd(nc, in_maps,
                                   core_ids=list(range(NCORES)), **kw)
    except Exception:
        # transient device wedges (e.g. NRT_EXEC_UNIT_UNRECOVERABLE) have
        # been observed to clear on a retry
        res = run_bass_kernel_spmd(nc, in_maps,
                                   core_ids=list(range(NCORES)), **kw)

    out = np.zeros((B, S, D), dtype=np.float32)
    for c in range(NCORES):
        out[c // 4] += res.results[c]["outp"]
    out += bo[None, None, :]
    return out, res


def kernel(**inputs) -> np.ndarray:
    out, _ = run(inputs)
    return out


# revision 37
# speedup vs baseline: 1.0039x; 1.0039x over previous
"""Multi-head attention TRN2 kernel (Bass/Tile), 8-core tensor-parallel.

Sharding: core c -> batch b=c//4, head group g=c%4 (4 heads = 256 features).
Host pre-transposes x and weight slices to bf16 and RELAYS them so each
SBUF partition's slice is one contiguous DRAM run (the DMA engines here
are descriptor-rate-bound, not bandwidth-bound): x as [128p, ts, e, 512]
(8KB/partition/slice), weights as [128p, e, 256] (4KB/partition).

Device computes qT/kT (features x tokens) and v in token-major
[v(64)|ones(64)] blocks per head, so each AV matmul emits the softmax
denominator replicated across psum partitions 64:128, row-aligned with the
context rows.  Causal softmax uses unnormalized exp (scalar engine) with
per-chunk column trimming at the diagonal; 1/denominator is exp(-ln(x)) on
the scalar engine, then one vector multiply.  The output projection
partial goes back fp32; the host sums 4 partials per batch and adds bias.

All matmul operands are bf16 (fp32 psum accumulation).  Pipeline: per
512-token slice: projections -> attention (heads paired per feature tile,
AV of group gi-1 issued between QK and exp of gi) -> output projection of
slice ts-1 emitted after attention(ts) so its matmuls fill the
activation-bound attention tail; stores batched [128,1024] and rotated
across the sync/scalar/gpsimd DMA queues so they drain during compute.
"""

import numpy as np

B, S, D = 2, 2048, 1024
H, HD = 16, 64
NCORES = 8
HPC = 4              # heads per core
FPC = HPC * HD       # 256 features per core
NF = FPC // 128      # 2 feature tiles of 128
KC = D // 128        # 8 contraction chunks
NTS = S // 512       # 4 token slices == q-tiles
NTT = S // 128       # 16 token tiles of 128
SCALE = 1.0 / 8.0    # 1/sqrt(HD)

_CACHE = {}


def _legalize_waits(nc, mybir, max_waits=1):
    """Walrus codegen allows only 1 sync-wait slot on most TPB instructions.
    Hoist extra waits into same-engine NoOps inserted just before."""
    n_fixed = 0
    for _, bb_wrap in nc.bb_map.items():
        bb = bb_wrap.bb
        out = []
        changed = False
        for inst in list(bb.instructions):
            si = inst.sync_info
            if si is not None and si.on_wait and len(si.on_wait) > max_waits:
                for w in list(si.on_wait[:-max_waits]):
                    nop = mybir.InstNoOp(
                        name=f"I-lw-{nc.next_id()}", engine=inst.engine,
                        ins=[], outs=[],
                        sync_info=mybir.SyncInfo(on_wait=[w], on_update=[]),
                    )
                    nop.text_hint = "dep"
                    out.append(nop)
                si.on_wait = list(si.on_wait[-max_waits:])
                n_fixed += 1
                changed = True
            out.append(inst)
        if changed:
            bb.instructions = out
    return n_fixed


def _build():
    import concourse.bass as bass
    import concourse.mybir as mybir
    from concourse.tile import TileContext
    from concourse.masks import make_upper_triangular

    F32 = mybir.dt.float32
    BF16 = mybir.dt.bfloat16
    EXP = mybir.ActivationFunctionType.Exp
    LN = mybir.ActivationFunctionType.Ln
    MUL = mybir.AluOpType.mult

    nc = bass.Bass()
    # host-relaid: [128p, ts, e, 512] flattened
    xT = nc.dram_tensor("xT", [128, NTS * KC * 512], BF16,
                        kind="ExternalInput")
    # host-relaid: [128p, e, 256] flattened
    wqT = nc.dram_tensor("wqT", [128, KC * FPC], BF16, kind="ExternalInput")
    wkT = nc.dram_tensor("wkT", [128, KC * FPC], BF16, kind="ExternalInput")
    wvT = nc.dram_tensor("wvT", [128, KC * FPC], BF16, kind="ExternalInput")
    woT = nc.dram_tensor("woT", [FPC, D], BF16, kind="ExternalInput")
    outp = nc.dram_tensor("outp", [S, D], BF16, kind="ExternalOutput")

    with TileContext(nc) as tc:
        with (
            tc.tile_pool(name="res", bufs=1) as res,
            tc.tile_pool(name="xp", bufs=2) as xp,
            tc.tile_pool(name="wk", bufs=4) as wkp,
            tc.tile_pool(name="osb", bufs=6) as osb,
            tc.tile_pool(name="pjps", bufs=1, space="PSUM") as pjps,
            tc.tile_pool(name="qkps", bufs=2, space="PSUM") as qkps,
            tc.tile_pool(name="smps", bufs=1, space="PSUM") as smps,
            tc.tile_pool(name="avps", bufs=1, space="PSUM") as avps,
        ):
            # ---- resident tensors -------------------------------------
            qT = [res.tile([128, S], BF16, name=f"qT{f}", tag=f"qT{f}")
                  for f in range(NF)]
            kT = [res.tile([128, S], BF16, name=f"kT{f}", tag=f"kT{f}")
                  for f in range(NF)]
            ctxT = [res.tile([128, S], BF16, name=f"ctxT{f}", tag=f"ctxT{f}")
                    for f in range(NF)]
            # v chunk layout: per head hh a [v(64) | ones(64)] block, so the
            # AV matmul emits the softmax denominator replicated over
            # partitions 64:128 of its psum bank.
            v_sb = [res.tile([128, 512], BF16, name=f"v{ck}", tag=f"v{ck}")
                    for ck in range(NTT)]
            woT_sb = [res.tile([128, D], BF16, name=f"wo{ic}", tag=f"wo{ic}")
                      for ic in range(NF)]

            # PE warm-up: a dense burst of dummy matmuls during the initial
            # DMA wait keeps the PE p-state at full rate until the first
            # projection group lands.  memset on the (empty) vector queue so
            # the burst starts before the DMA programming below.
            warm_sb = res.tile([128, 512], BF16, name="warm")
            nc.vector.memset(warm_sb, 0.0)
            wps = pjps.tile([128, 512], F32, name="pqk", tag="sm")
            for wi in range(32):
                c0 = 256 * (wi % 2)
                nc.tensor.matmul(wps[:, c0:c0 + 256], warm_sb[:, 0:128],
                                 warm_sb[:, 0:256], start=True, stop=True)

            # Batched loads: one DMA per tensor slice; the host layout makes
            # each partition's slice a single contiguous DRAM run, so the
            # descriptor count per load is 128 (vs 1k+ for strided views).
            _dmae = [nc.sync, nc.scalar, nc.gpsimd]
            # store/x-load queues: never scalar (it owns the exp stream)
            _dmst = [nc.sync, nc.gpsimd]

            def load_x(ts):
                t = xp.tile([128, KC * 512], BF16, name="x", tag="x")
                _dmst[ts % 2].dma_start(
                    out=t, in_=xT[:, ts * KC * 512:(ts + 1) * KC * 512])
                return [t[:, e * 512:(e + 1) * 512] for e in range(KC)]

            w_t = {}

            def load_w(nm, dram, eng):
                t = wkp.tile([128, KC * FPC], BF16, name=f"w{nm}",
                             tag=f"w{nm}", bufs=1)
                eng.dma_start(out=t, in_=dram[:, :])
                for e in range(KC):
                    w_t[nm, e] = t[:, e * FPC:(e + 1) * FPC]

            xch0 = load_x(0)
            load_w("q", wqT, nc.scalar)
            load_w("k", wkT, nc.gpsimd)
            load_w("v", wvT, nc.sync)

            # constants
            ones_f = res.tile([128, 64], F32)
            nc.gpsimd.memset(ones_f, 1.0)
            mask_f = res.tile([128, 128], F32)
            make_upper_triangular(nc, mask_f, val=1.0, diag=True)
            mask_r = res.tile([128, 128], BF16)
            nc.vector.tensor_copy(mask_r, mask_f)
            for ck in range(NTT):
                v4 = v_sb[ck].rearrange("p (h c) -> p h c", c=128)
                for hh in range(HPC):
                    nc.vector.tensor_copy(v4[:, hh, 64:128], ones_f)

            for ic in range(NF):
                nc.sync.dma_start(
                    out=woT_sb[ic],
                    in_=woT[ic * 128:(ic + 1) * 128, :])

            # Output-projection groups double as PE bubble fillers: their
            # matmuls depend only on already-normalized ctx slices, so they
            # are emitted into the attention's per-f pipeline-fill/drain
            # bubbles.  Stores are batched per token tile ([128,1024] = one
            # contiguous 2KB run per partition) and rotated across the
            # sync/scalar/gpsimd DMA queues so they drain during compute.
            def op_groups(tts, pools, eng=None):
                for gidx, tt in enumerate(tts):
                    so = osb.tile([128, D], BF16, name="so", tag="so")
                    for os_ in range(2):
                        pool, tg = pools[(gidx * 2 + os_) % len(pools)]
                        po = pool.tile([128, 512], F32, name="po", tag=tg)
                        for ic in range(NF):
                            nc.tensor.matmul(
                                po,
                                ctxT[ic][:, tt * 128:(tt + 1) * 128],
                                woT_sb[ic][:, os_ * 512:(os_ + 1) * 512],
                                start=(ic == 0), stop=(ic == NF - 1))
                        nc.vector.tensor_copy(
                            so[:, os_ * 512:(os_ + 1) * 512], po)
                    (eng or nc.sync).dma_start(
                        out=outp[tt * 128:(tt + 1) * 128, :], in_=so)

            MIDP = [(smps, "po"), (avps, "av")]
            ENDP = [(smps, "po"), (qkps, "qk")]

            # Projection emitters: each returns a closure emitting one
            # psum-group (8 matmuls + drain copy).  For slice 0 they run
            # as a block before attention(0); for slice ts+1 they are
            # interleaved INTO attention(ts)'s gi loop as PE filler — the
            # attention is scalar(exp)-bound, so the PE has ~0.7us of
            # slack per gi that would otherwise downclock the clock gate
            # (HAM) and stretch every matmul.
            _pj = [(pjps, "sm"), (smps, "po")]

            def proj_emitters(ts, xch):
                ems = []
                pji = [0]

                def qk_group(nm, dst, f):
                    def emit():
                        pool, tg = _pj[pji[0] % 2]
                        pji[0] += 1
                        ps = pool.tile([128, 512], F32, name="pqk", tag=tg)
                        for e in range(KC):
                            nc.tensor.matmul(
                                ps, w_t[nm, e][:, f * 128:(f + 1) * 128],
                                xch[e], start=(e == 0), stop=(e == KC - 1))
                        nc.vector.tensor_copy(
                            dst[f][:, ts * 512:(ts + 1) * 512], ps)
                    return emit

                def v_group(tt):
                    def emit():
                        ck = ts * 4 + tt
                        pool, tg = _pj[pji[0] % 2]
                        pji[0] += 1
                        ps = pool.tile([128, FPC], F32, name="pv", tag=tg)
                        for e in range(KC):
                            nc.tensor.matmul(
                                ps, xch[e][:, tt * 128:(tt + 1) * 128],
                                w_t["v", e],
                                start=(e == 0), stop=(e == KC - 1))
                        v4 = v_sb[ck].rearrange("p (h c) -> p h c", c=128)
                        ps4 = ps.rearrange("p (g c) -> p g c", c=64)
                        nc.vector.tensor_copy(v4[:, :, 0:64], ps4)
                    return emit

                for nm, dst in (("q", qT), ("k", kT)):
                    for f in range(NF):
                        ems.append(qk_group(nm, dst, f))
                for tt in range(4):
                    ems.append(v_group(tt))
                return ems

            # ---- pipelined slices -------------------------------------
            xch_next = xch0
            for ts in range(NTS):
                xch = xch_next
                if ts == 0:
                    for em in proj_emitters(0, xch):
                        em()
                # prefetch + queue next slice's projections as filler
                if ts + 1 < NTS:
                    xch_next = load_x(ts + 1)
                    fillers = proj_emitters(ts + 1, xch_next)
                else:
                    fillers = []

                # attention for q-tile j == ts.  The two heads of each
                # feature tile f run as a row-tiled pair: head hl=0 on PE
                # rows 0:64 (tile_position (0,0)), hl=1 on rows 64:128
                # ((64,0)) — concurrent on the array, halving QK^T time.
                # AV matmuls of group gi-1 are issued between the QK pair
                # of gi and its exp so the PE stays busy during the
                # activation.
                j = ts
                sq0 = 512 * j
                ndiag = 4 * j  # first diagonal chunk index

                def qk_trim(ci):
                    """Leading fully-masked columns to skip."""
                    return max(128 * ci - sq0, 0)

                for f in range(NF):
                    av = avps.tile([128, 1024], F32, name="av", tag="av")
                    ag_prev = None

                    def av_mms(ag_pair, gi):
                        for slot in range(2):
                            ci = 2 * gi + slot
                            tr = qk_trim(ci)
                            for hl in range(2):
                                hh = 2 * f + hl
                                nc.tensor.matmul(
                                    av[:, 512 * hl + tr:512 * hl + 512],
                                    v_sb[ci][:, 128 * hh:128 * hh + 128],
                                    ag_pair[hl][:, slot * 512 + tr:
                                                 slot * 512 + 512],
                                    start=(ci == 0),
                                    stop=(ci == 4 * j + 3))

                    for gi in range(2 * j + 2):
                        qk_pair = [qkps.tile([128, 1024], F32, name="qk",
                                             tag="qk") for _ in range(2)]
                        for slot in range(2):
                            ci = 2 * gi + slot
                            tr = qk_trim(ci)
                            for hl in range(2):
                                r0 = 64 * hl
                                nc.tensor.matmul(
                                    qk_pair[hl][:, slot * 512 + tr:
                                                slot * 512 + 512],
                                    kT[f][r0:r0 + 64,
                                          ci * 128:(ci + 1) * 128],
                                    qT[f][r0:r0 + 64,
                                          sq0 + tr:sq0 + 512],
                                    start=True, stop=True)
                        if ag_prev is not None:
                            av_mms(ag_prev, gi - 1)
                        # pace one projection filler group per two gi
                        # iterations: per gi the scalar engine needs ~2us
                        # of exp while the attention's own matmuls cover
                        # only ~1.3us; the filler absorbs the difference.
                        if gi % 2 == 1 and fillers:
                            fillers.pop(0)()
                        ag_pair = [osb.tile([128, 1024], BF16, name="ag",
                                            tag="ag", bufs=6)
                                   for _ in range(2)]
                        for hl in range(2):
                            qk, ag = qk_pair[hl], ag_pair[hl]
                            if gi < 2 * j:  # fully-valid chunks
                                nc.scalar.activation(ag, qk, EXP,
                                                     scale=SCALE)
                            else:
                                # diagonal: one exp from the first valid
                                # column onward (cols the av matmuls never
                                # read may hold garbage), then triangular
                                # masks per 128-chunk.
                                tr0 = qk_trim(2 * gi)
                                nc.scalar.activation(
                                    ag[:, tr0:1024], qk[:, tr0:1024],
                                    EXP, scale=SCALE)
                                for slot in range(2):
                                    ci = 2 * gi + slot
                                    dlt = 128 * ci - sq0
                                    w0 = slot * 512
                                    nc.vector.tensor_tensor(
                                        ag[:, w0 + dlt:w0 + dlt + 128],
                                        ag[:, w0 + dlt:w0 + dlt + 128],
                                        mask_r, MUL)
                        ag_prev = ag_pair
                    av_mms(ag_prev, 2 * j + 1)

                    # normalize: ctxT = av[0:64] * (1/av[64:128]); the ones
                    # half of each v block put the denominator in av rows
                    # 64:128 (replicated), row-aligned with ctx.  1/x is
                    # exp(-ln(x)) on the scalar engine: ln and exp share one
                    # activation table.  Both heads batched per call.
                    lns = osb.tile([64, 1024], F32, name="lns", tag="lns")
                    nc.scalar.activation(lns, av[64:128, :], LN)
                    rcp = osb.tile([64, 1024], F32, name="rcp", tag="rcp")
                    nc.scalar.activation(rcp, lns, EXP, scale=-1.0)
                    for hl in range(2):
                        r0 = 64 * hl
                        nc.vector.tensor_tensor(
                            ctxT[f][r0:r0 + 64, sq0:sq0 + 512],
                            av[0:64, 512 * hl:512 * hl + 512],
                            rcp[:, 512 * hl:512 * hl + 512], MUL)

                    # mid-slice filler: cover the f=0 drain / f=1 fill
                    # bubble with ready out-projection groups.
                    if f == 0 and ts >= 1:
                        op_groups([4 * (ts - 1), 4 * (ts - 1) + 1], MIDP)

                # leftover projection groups not consumed by the gi loop
                for em in fillers:
                    em()
                fillers = []

                # end-of-slice filler: cover the f=1 drain bubble.
                if ts >= 2:
                    op_groups([4 * (ts - 1) + 2, 4 * (ts - 1) + 3], ENDP)

            # Tail: op0's second half is dependency-free PE work that runs
            # while the final normalize chain (scalar ln/exp + vector
            # muls) drains; then op3, whose ic=1 matmuls are emitted after
            # all ic=0 so the PE chews on those first if ctxT[1] is late.
            op_groups([2, 3], ENDP)
            op3_pools = [(smps, "po"), (pjps, "sm"), (qkps, "qk"),
                         (qkps, "qk")]
            for wave in range(2):
                tts = [12 + 2 * wave, 13 + 2 * wave]
                pos = {}
                for wi, tt in enumerate(tts):
                    for os_ in range(2):
                        pool, tg = op3_pools[(2 * wi + os_) % 4]
                        po = pool.tile([128, 512], F32, name="po", tag=tg)
                        pos[tt, os_] = po
                        nc.tensor.matmul(
                            po, ctxT[0][:, tt * 128:(tt + 1) * 128],
                            woT_sb[0][:, os_ * 512:(os_ + 1) * 512],
                            start=True, stop=False)
                for tt in tts:
                    so = osb.tile([128, D], BF16, name="so", tag="so")
                    for os_ in range(2):
                        po = pos[tt, os_]
                        nc.tensor.matmul(
                            po, ctxT[1][:, tt * 128:(tt + 1) * 128],
                            woT_sb[1][:, os_ * 512:(os_ + 1) * 512],
                            start=False, stop=True)
                        nc.vector.tensor_copy(
                            so[:, os_ * 512:(os_ + 1) * 512], po)
                    nc.sync.dma_start(
                        out=outp[tt * 128:(tt + 1) * 128, :], in_=so)

    _legalize_waits(nc, mybir)
    return nc


def _prep_inputs(in_data, Wq, Wk, Wv, Wo):
    import ml_dtypes

    bf16 = ml_dtypes.bfloat16
    in_maps = []
    for c in range(NCORES):
        b, g = c // 4, c % 4
        sl = slice(g * FPC, (g + 1) * FPC)
        # x: [D, S] -> [128p, ts, e, 512] so each (p, ts) is one
        # contiguous 8KB DRAM run.
        xt = np.ascontiguousarray(in_data[b].T).astype(bf16)
        xr = np.ascontiguousarray(
            xt.reshape(KC, 128, NTS, 512).transpose(1, 2, 0, 3)
        ).reshape(128, NTS * KC * 512)

        def wrelay(W):
            # [D, FPC] -> [128p, e, FPC] contiguous per partition
            wt = np.ascontiguousarray(W[sl, :].T).astype(bf16)
            return np.ascontiguousarray(
                wt.reshape(KC, 128, FPC).transpose(1, 0, 2)
            ).reshape(128, KC * FPC)

        in_maps.append({
            "xT": xr,
            "wqT": wrelay(Wq),
            "wkT": wrelay(Wk),
            "wvT": wrelay(Wv),
            "woT": np.ascontiguousarray(Wo[:, sl].T).astype(bf16),
        })
    return in_maps


def run(inputs, trace=False):
    from concourse.bass_utils import run_bass_kernel_spmd

    in_data = np.asarray(inputs["in_data"], dtype=np.float32)
    Wq = np.asarray(inputs["Wq"], dtype=np.float32)
    Wk = np.asarray(inputs["Wk"], dtype=np.float32)
    Wv = np.asarray(inputs["Wv"], dtype=np.float32)
    Wo = np.asarray(inputs["Wo"], dtype=np.float32)
    bo = np.asarray(inputs["bo"], dtype=np.float32)

    if "nc" not in _CACHE:
        _CACHE["nc"] = _build()
    nc = _CACHE["nc"]

    in_maps = _prep_inputs(in_data, Wq, Wk, Wv, Wo)
    kw = {}
    if trace:
        kw = dict(trace=True, trace_cores=list(range(NCORES)))
    try:
        res = run_bass_kernel_spmd(nc, in_maps,
                                   core_ids=list(range(NCORES)), **kw)
    except Exception:
        # transient device wedges (e.g. NRT_EXEC_UNIT_UNRECOVERABLE) have
        # been observed to clear on a retry
        res = run_bass_kernel_spmd(nc, in_maps,
                                   core_ids=list(range(NCORES)), **kw)

    out = np.zeros((B, S, D), dtype=np.float32)
    for c in range(NCORES):
        out[c // 4] += res.results[c]["outp"]
    out += bo[None, None, :]
    return out, res


def kernel(**inputs) -> np.ndarray:
    out, _ = run(inputs)
    return out
